# revision 7
# baseline (speedup 1.0000x reference)
"""Trainium2 Bass kernel for CrossModalFusion (B=4, C=64, H=W=64, N=4096).

Reference computation (per sample b, with x reshaped to [C, N]):
    q = wq @ xo + bq          [8, N]
    k = wk @ xs + bk          [8, N]
    v = wv @ xs + bv          [64, N]
    S[n, m]  = q[:, n] . k[:, m]
    attn     = softmax_m(S)
    out      = gamma * (v @ attn^T) + x_opt

Sharding: 8 cores = 4 batch samples x 2 halves of the query (n) axis.
Each core computes output rows [64, 2048] for its (sample, n-half); no
cross-core communication is needed.

Per-core dataflow:
  - biases are folded into augmented weights on the host (ones-row trick),
    so q/k/v come out of single matmuls against xs_aug/xo_aug ([65, *]
    tiles whose last row is 1.0).
  - scores are computed TRANSPOSED (S^T[m, n]) so that the exp'd scores can
    feed the attention*V matmul directly as the moving operand.  v^T gets an
    extra ones column, so the AV matmul's output row 64 accumulates
    sum_m exp(S[n, m]) — the softmax denominator comes out of the same
    accumulation for free.  No max-subtraction is needed: scores are O(3).
  - q/k are replicated at partition offsets 0 and 64 so the rank-8 S^T
    matmuls run two-at-a-time in distinct PE row groups.
  - per n-tile of 512, accumulate over all 32 m-blocks, then normalize by
    1/denominator, scale by gamma, add the x_opt residual and DMA out.
"""

import os
import sys

import numpy as np

for _p in ("/opt/trn_rl_repo", "/root/.axon_site/_ro/trn_rl_repo"):
    if os.path.isdir(_p) and _p not in sys.path:
        sys.path.insert(0, _p)

import concourse.bass as bass
import concourse.mybir as mybir
import concourse.tile as tile
from concourse import bacc
from concourse.bass_utils import run_bass_kernel_spmd

F32 = mybir.dt.float32
AF = mybir.ActivationFunctionType

B, C, HH, WW = 4, 64, 64, 64
N = HH * WW            # 4096 key/query positions
D = 8                  # q/k channel count
CA = C + 1             # augmented channel dim (ones row / denominator row)
NCORES = 8
NL = N // 2            # query rows per core
NT = 512               # n-tile (PSUM bank width in fp32)
MB = 128               # m-block (PE partition width)
N_NT = NL // NT        # 4 n-tiles per core
N_MB = N // MB         # 32 m-blocks
WAVE = 2               # m-blocks exp'd per ACT instruction


def build_program() -> bass.Bass:
    # Bacc (not raw Bass): its compile() pass splits multi-semaphore waits
    # and moves matmul waits onto LDWEIGHTS, which this walrus build requires.
    nc = bacc.Bacc("TRN2", target_bir_lowering=False, num_devices=NCORES)
    # xo/xs arrive host-augmented with a trailing ones row ([65, *]) so PE
    # matmuls only wait on DMA producers (PE LDWEIGHTS allows max 2 sync
    # waits; an extra on-chip memset producer pushed it to 3).
    xo_d = nc.declare_dram_parameter("xo_aug", [CA, NL], F32, isOutput=False)
    xs_d = nc.declare_dram_parameter("xs_aug", [CA, N], F32, isOutput=False)
    wq_d = nc.declare_dram_parameter("wq_aug", [CA, D], F32, isOutput=False)
    wk_d = nc.declare_dram_parameter("wk_aug", [CA, D], F32, isOutput=False)
    wv_d = nc.declare_dram_parameter("wv_aug", [CA, CA], F32, isOutput=False)
    g_d = nc.declare_dram_parameter("gamma", [1, 1], F32, isOutput=False)
    out_d = nc.declare_dram_parameter("out", [C, NL], F32, isOutput=True)

    with tile.TileContext(nc) as tc:
        with tc.tile_pool(name="const", bufs=1) as cp:
            wq_sb = cp.tile([CA, D], F32)
            nc.sync.dma_start(wq_sb[:], wq_d[:])
            wk_sb = cp.tile([CA, D], F32)
            nc.sync.dma_start(wk_sb[:], wk_d[:])
            wv_sb = cp.tile([CA, CA], F32)
            nc.sync.dma_start(wv_sb[:], wv_d[:])
            g_sb = cp.tile([1, 1], F32)
            nc.sync.dma_start(g_sb[:], g_d[:])
            ones_sb = cp.tile([1, C], F32)
            nc.vector.memset(ones_sb[:], 1.0)

            xs_aug = cp.tile([CA, N], F32)
            for j in range(4):
                nc.sync.dma_start(
                    xs_aug[:, j * 1024 : (j + 1) * 1024],
                    xs_d[:, j * 1024 : (j + 1) * 1024],
                )

            xo_aug = cp.tile([CA, NL], F32)
            for j in range(2):
                nc.sync.dma_start(
                    xo_aug[:, j * 1024 : (j + 1) * 1024],
                    xo_d[:, j * 1024 : (j + 1) * 1024],
                )

            # q/k at partition offsets 0 and 64 (PE row groups for the
            # concurrent rank-8 score matmuls); vT augmented with ones col.
            q_rep = cp.tile([64 + D, NL], F32)
            k_rep = cp.tile([64 + D, N], F32)
            vT = cp.tile([MB, N_MB * CA], F32)

            with tc.tile_pool(name="pre_ps", bufs=2, space="PSUM") as pp:
                for j in range(N_NT):
                    qp = pp.tile([D, NT], F32, tag="qk_ps")
                    nc.tensor.matmul(
                        qp[:], wq_sb[:], xo_aug[:, j * NT : (j + 1) * NT],
                        start=True, stop=True,
                    )
                    nc.vector.tensor_copy(q_rep[0:D, j * NT : (j + 1) * NT], qp[:])
                    nc.sync.dma_start(
                        q_rep[64 : 64 + D, j * NT : (j + 1) * NT],
                        q_rep[0:D, j * NT : (j + 1) * NT],
                    )
                for j in range(N // NT):
                    kp = pp.tile([D, NT], F32, tag="qk_ps")
                    nc.tensor.matmul(
                        kp[:], wk_sb[:], xs_aug[:, j * NT : (j + 1) * NT],
                        start=True, stop=True,
                    )
                    nc.vector.tensor_copy(k_rep[0:D, j * NT : (j + 1) * NT], kp[:])
                    nc.sync.dma_start(
                        k_rep[64 : 64 + D, j * NT : (j + 1) * NT],
                        k_rep[0:D, j * NT : (j + 1) * NT],
                    )
                # v^T blocks: [128, 65] = xs_aug-block^T @ wv_aug.  Column 64
                # is all-ones (denominator column) since xs_aug row 64 is 1.
                for mb in range(N_MB):
                    vp = pp.tile([MB, CA], F32, tag="vt_ps")
                    nc.tensor.matmul(
                        vp[:], xs_aug[:, mb * MB : (mb + 1) * MB], wv_sb[:],
                        start=True, stop=True,
                    )
                    nc.vector.tensor_copy(vT[:, mb * CA : (mb + 1) * CA], vp[:])

            with (
                tc.tile_pool(name="st_ps", bufs=2, space="PSUM") as st_pool,
                tc.tile_pool(name="av_ps", bufs=2, space="PSUM") as av_pool,
                tc.tile_pool(name="bc_ps", bufs=2, space="PSUM") as bc_pool,
                tc.tile_pool(name="e_sb", bufs=4) as e_pool,
                tc.tile_pool(name="o_sb", bufs=3) as o_pool,
                tc.tile_pool(name="sm_sb", bufs=3) as sm_pool,
            ):
                for nt in range(N_NT):
                    n0, n1 = nt * NT, (nt + 1) * NT
                    av = av_pool.tile([CA, NT], F32)

                    def emit_av(e_t, w, av=av):
                        for j in range(WAVE):
                            mb = WAVE * w + j
                            nc.tensor.matmul(
                                av[:],
                                vT[:, mb * CA : (mb + 1) * CA],
                                e_t[:, j * NT : (j + 1) * NT],
                                start=(mb == 0),
                                stop=(mb == N_MB - 1),
                            )

                    # S^T matmuls + exp, with the AV accumulation lagging one
                    # wave so the PE never stalls waiting on the current exp.
                    pend = None
                    for w in range(N_MB // WAVE):
                        st = st_pool.tile([MB, WAVE * NT], F32)
                        for j in range(WAVE):
                            mb = WAVE * w + j
                            rg = 64 * j
                            nc.tensor.matmul(
                                st[:, j * NT : (j + 1) * NT],
                                k_rep[rg : rg + D, mb * MB : (mb + 1) * MB],
                                q_rep[rg : rg + D, n0:n1],
                                start=True,
                                stop=True,
                            )
                        e_t = e_pool.tile([MB, WAVE * NT], F32)
                        nc.scalar.activation(e_t[:], st[:], AF.Exp)
                        if pend is not None:
                            emit_av(*pend)
                        pend = (e_t, w)
                    emit_av(*pend)

                    # normalize: out = gamma/denom * unnorm + x_opt
                    recip = sm_pool.tile([1, NT], F32, tag="recip")
                    nc.vector.reciprocal(recip[:], av[C:CA, :])
                    sr = sm_pool.tile([1, NT], F32, tag="sr")
                    nc.vector.tensor_scalar_mul(sr[:], recip[:], g_sb[0:1, 0:1])
                    bc = bc_pool.tile([C, NT], F32)
                    nc.tensor.matmul(bc[:], ones_sb[:], sr[:], start=True, stop=True)
                    bcs = o_pool.tile([C, NT], F32, tag="bcs")
                    nc.vector.tensor_copy(bcs[:], bc[:])
                    om = o_pool.tile([C, NT], F32, tag="om")
                    nc.vector.tensor_mul(om[:], av[0:C, :], bcs[:])
                    o = o_pool.tile([C, NT], F32, tag="o")
                    nc.vector.tensor_add(o[:], om[:], xo_aug[0:C, n0:n1])
                    nc.sync.dma_start(out_d[:, n0:n1], o[:])
    nc.compile()
    return nc


_NC = None


def _get_nc() -> bass.Bass:
    global _NC
    if _NC is None:
        _NC = build_program()
    return _NC


def make_in_maps(x_opt, x_sar, wq, bq, wk, bk, wv, bv, gamma):
    f = np.float32
    x_opt = np.asarray(x_opt, f).reshape(B, C, N)
    x_sar = np.asarray(x_sar, f).reshape(B, C, N)
    wq_aug = np.ascontiguousarray(
        np.concatenate([np.asarray(wq, f).T, np.asarray(bq, f)[None, :]], axis=0)
    )
    wk_aug = np.ascontiguousarray(
        np.concatenate([np.asarray(wk, f).T, np.asarray(bk, f)[None, :]], axis=0)
    )
    wv_aug = np.zeros((CA, CA), f)
    wv_aug[:C, :C] = np.asarray(wv, f).T
    wv_aug[C, :C] = np.asarray(bv, f)
    wv_aug[C, C] = 1.0
    g = np.asarray(gamma, f).reshape(1, 1)
    ones_n = np.ones((1, N), f)
    maps = []
    for core in range(NCORES):
        b, h = divmod(core, 2)
        xo_aug = np.concatenate(
            [x_opt[b, :, h * NL : (h + 1) * NL], ones_n[:, :NL]], axis=0
        )
        xs_aug = np.concatenate([x_sar[b], ones_n], axis=0)
        maps.append(
            {
                "xo_aug": np.ascontiguousarray(xo_aug),
                "xs_aug": np.ascontiguousarray(xs_aug),
                "wq_aug": wq_aug,
                "wk_aug": wk_aug,
                "wv_aug": wv_aug,
                "gamma": g,
            }
        )
    return maps


def assemble_out(results) -> np.ndarray:
    out = np.empty((B, C, N), np.float32)
    for core in range(NCORES):
        b, h = divmod(core, 2)
        out[b, :, h * NL : (h + 1) * NL] = results[core]["out"]
    return out.reshape(B, C, HH, WW)


def kernel(**inputs) -> np.ndarray:
    nc = _get_nc()
    maps = make_in_maps(**inputs)
    res = run_bass_kernel_spmd(nc, maps, list(range(NCORES)))
    return assemble_out(res.results)
